# revision 13
# baseline (speedup 1.0000x reference)
"""Multi-head causal attention on 8 TRN2 NeuronCores.

Sharding: (batch, head-group) across 8 cores — core c handles batch c//4 and
heads [4*(c%4), 4*(c%4)+4). After attention, an 8-rank AllToAll exchanges
per-head attention outputs so core c computes the final output projection for
rows [512*(c%4), 512*(c%4)+512) of batch c//4. Host-side unshard is a pure
concatenation.

All matmuls run in bf16 (fp32 PSUM accumulation). Softmax is computed without
max-subtraction (scores*scale are O(1) for these inputs); the denominator is
obtained by augmenting V with a ones column. The division uses the fast
approximate DVE reciprocal plus a GpSimd partition-broadcast and one vector
multiply — cheap enough to stay off the PE critical path.
"""
import numpy as np
import ml_dtypes

B, S, D, H = 2, 2048, 1024, 16
DH = D // H          # 64
DIM_K = 1024
NCORES = 8
HC = 4               # heads per core
C = HC * DH          # 256 dh-columns per core
NQC = 4              # q-chunks of 512
QCH = 512
NKT = 16             # k-tiles of 128
NDC = 8              # d-chunks of 128
SCALE = float(DIM_K) ** -0.5  # 1/32

_cache = {}


def _emit_body(nc, tc, pools, ins, it):
    """Emit one full kernel body (iteration `it` for duplication timing)."""
    import concourse.bass as bass
    from concourse import mybir

    f32 = mybir.dt.float32
    f32r = mybir.dt.float32r
    bf16 = mybir.dt.bfloat16
    EXP = mybir.ActivationFunctionType.Exp
    LN = mybir.ActivationFunctionType.Ln

    persist, exps, aop, recips, osb, ps_big, ps_av, dram = pools
    x_in, wq_in, wk_in, wv_in, wo_in, tri_in, info_in, out = ins

    # ---------------- Phase A: loads ----------------
    # Host passes partition-major layouts, so every load is one contiguous
    # DMA. Small QKV weights first (they gate the first matmuls); x comes in
    # per-512-column chunks so chunk-0 projections can start early; Wo last.
    wq_sb = persist.tile([128, NDC, C], bf16, name=f"wq_sb_{it}", tag="wq_sb")
    wk_sb = persist.tile([128, NDC, C], bf16, name=f"wk_sb_{it}", tag="wk_sb")
    wv_sb = persist.tile([128, NDC, C], bf16, name=f"wv_sb_{it}", tag="wv_sb")
    wo_sb = persist.tile([128, NDC, DIM_K], bf16, name=f"wo_sb_{it}", tag="wo_sb")
    nc.sync.dma_start(out=wq_sb[:], in_=wq_in.ap())
    nc.sync.dma_start(out=wk_sb[:], in_=wk_in.ap())
    nc.sync.dma_start(out=wv_sb[:], in_=wv_in.ap())

    xT = [persist.tile([128, S], bf16, name=f"xT{j}_{it}", tag=f"xT{j}")
          for j in range(NDC)]
    for j in range(NDC):
        nc.sync.dma_start(out=xT[j][:, 0:QCH],
                          in_=x_in[128 * j:128 * (j + 1), 0:QCH])

    tri = persist.tile([128, 128], bf16, name=f"tri_{it}", tag="tri")
    nc.sync.dma_start(out=tri[:], in_=tri_in.ap())

    for qc in range(1, NQC):
        for j in range(NDC):
            nc.sync.dma_start(
                out=xT[j][:, QCH * qc:QCH * (qc + 1)],
                in_=x_in[128 * j:128 * (j + 1), QCH * qc:QCH * (qc + 1)])

    nc.sync.dma_start(out=wo_sb[:], in_=wo_in.ap())

    # ---------------- Phase B: QKV projections ----------------
    # Q^T / K^T in pair tiles: [128, S], heads (2p, 2p+1) at partitions
    # [0,64) / [64,128).
    qt, kt = [None, None], [None, None]

    def emit_qtkt(p):
        qtp = persist.tile([128, S], bf16, name=f"qt{p}_{it}", tag=f"qt{p}")
        ktp = persist.tile([128, S], bf16, name=f"kt{p}_{it}", tag=f"kt{p}")
        qt[p] = qtp
        kt[p] = ktp
        for w_sb, dst in ((wq_sb, qtp), (wk_sb, ktp)):
            for qc in range(NQC):
                ps = ps_big.tile([128, QCH], f32, tag="big",
                                 name=f"qkps{p}_{qc}_{w_sb.name[:2]}_{it}")
                for j in range(NDC):
                    nc.tensor.matmul(
                        ps[:],
                        lhsT=w_sb[:, j, 128 * p:128 * (p + 1)],
                        rhs=xT[j][:, QCH * qc:QCH * (qc + 1)],
                        start=(j == 0), stop=(j == NDC - 1),
                    )
                nc.vector.tensor_copy(dst[:, QCH * qc:QCH * (qc + 1)], ps[:])

    emit_qtkt(0)

    # V padded to 128 columns: [ones | 63 zeros | 64 data] per head. The
    # ones column at index 0 puts the softmax denominator in PSUM row 0
    # (partition_broadcast can only read a partition-0 source) and the
    # data rows at partitions 64..127 (engine APs need aligned bases).
    # 128-wide bf16 weights also enable FWL fast weight loads on the PE.
    vp = []
    for i in range(NKT):
        t = persist.tile([128, HC, 2 * DH], bf16, name=f"vp{i}_{it}",
                         tag=f"vp{i}")
        nc.vector.memset(t[:, :, 0:DH], 0.0)
        nc.vector.memset(t[:, :, 0:1], 1.0)
        ps = ps_big.tile([128, C], f32, tag="big", name=f"vps{i}_{it}")
        for j in range(NDC):
            nc.tensor.matmul(
                ps[:],
                lhsT=xT[j][:, 128 * i:128 * (i + 1)],
                rhs=wv_sb[:, j, :],
                start=(j == 0), stop=(j == NDC - 1),
            )
        nc.vector.tensor_copy(
            t[:, :, DH:2 * DH], ps[:].rearrange("p (h d) -> p h d", h=HC))
        vp.append(t)

    # pair-1 projections emitted here so the scheduler can fill PE gaps
    # during pair-0's (ACT-bound) attention with these matmuls
    emit_qtkt(1)

    # ---------------- Phase C: attention ----------------
    # Per head-pair AllToAll buffers: block j carries my pair-p rows for
    # rank j's s-block. I fill only blocks [4b, 4b+4) (my batch's ranks);
    # 4b comes from coreinfo at runtime.
    blk = nc.gpsimd.alloc_register(f"blk_{it}")
    nc.gpsimd.reg_load(blk, info_in[0:1, 0:1])
    blk_sv = nc.gpsimd.snap(blk, donate=True, min_val=0, max_val=NCORES - HC)

    a2a_in = [dram.tile([NCORES, 128, QCH], bf16, name=f"a2a_in{p}_{it}",
                        tag=f"a2a_in{p}") for p in range(2)]
    a2a_out = [dram.tile([NCORES, 128, QCH], bf16, name=f"a2a_out{p}_{it}",
                         tag=f"a2a_out{p}") for p in range(2)]
    def emit_normalize(p, c, avs):
        for h2 in range(2):
            # 1/denom = exp(-ln(denom)) on the ACT LUT engine; the
            # broadcast across partitions runs on GpSimd. The PE and
            # DVE stay out of the softmax-normalize chain entirely.
            lnd = recips.tile([1, QCH], f32, tag="lnd",
                              name=f"lnd{p}_{c}_{h2}_{it}")
            nc.scalar.activation(out=lnd[:], in_=avs[h2][0:1, :],
                                 func=LN)
            rc = recips.tile([1, QCH], f32, tag="rc",
                             name=f"rc{p}_{c}_{h2}_{it}")
            nc.scalar.activation(out=rc[:], in_=lnd[:], func=EXP,
                                 scale=-1.0)
            bc_sb = recips.tile([128, QCH], f32, tag="bcsb",
                                name=f"bcsb{p}_{c}_{h2}_{it}")
            nc.gpsimd.partition_broadcast(bc_sb[:], rc[0:1, :])
            ao = aop.tile([128, QCH], bf16, tag="ao",
                          name=f"ao{p}_{c}_{h2}_{it}")
            nc.vector.tensor_mul(ao[DH:2 * DH, :],
                                 avs[h2][DH:2 * DH, :],
                                 bc_sb[DH:2 * DH, :])
            # static writes to both batches' candidate blocks (c, c+4);
            # the wrong-batch block is ignored by its receiver
            for bb in range(2):
                nc.sync.dma_start(
                    out=a2a_in[p][HC * bb + c, DH * h2:DH * (h2 + 1), :],
                    in_=ao[DH:2 * DH, :])

    for p in range(2):
        pending = None  # (c, avs) whose normalize is deferred 2 tiles
        for c in range(NQC):
            avs = [ps_av.tile([128, QCH], f32, tag="av",
                              name=f"av{p}_{c}_{i2}_{it}")
                   for i2 in range(2)]
            njt = 4 * c + 4
            for j in range(njt):
                off = max(0, 128 * j - QCH * c)
                sc = ps_big.tile([128, 2 * QCH], f32, tag="big",
                                 name=f"sc{p}_{c}_{j}_{it}")
                sc3 = sc[:].rearrange("p (h n) -> p h n", h=2)
                ex = exps.tile([128, 2, QCH], bf16, tag="ex",
                               name=f"ex{p}_{c}_{j}_{it}")
                for h2 in range(2):
                    nc.tensor.matmul(
                        sc3[:, h2, off:QCH],
                        lhsT=kt[p][64 * h2:64 * (h2 + 1), 128 * j:128 * (j + 1)],
                        rhs=qt[p][64 * h2:64 * (h2 + 1),
                                  QCH * c + off:QCH * (c + 1)],
                        start=True, stop=True,
                    )
                nc.scalar.activation(
                    out=ex[:, :, off:QCH], in_=sc3[:, :, off:QCH],
                    func=EXP, scale=SCALE)
                if j // 4 == c:
                    # diagonal tile: zero the strictly-lower triangle
                    nc.vector.tensor_mul(
                        ex[:, :, off:off + 128],
                        ex[:, :, off:off + 128],
                        tri[:].unsqueeze(1).to_broadcast([128, 2, 128]),
                    )
                for h2 in range(2):
                    nc.tensor.matmul(
                        avs[h2][:, off:QCH],
                        lhsT=vp[j][:, 2 * p + h2, :],
                        rhs=ex[:, h2, off:QCH],
                        start=(j == 0), stop=(j == njt - 1),
                    )
                if pending is not None and j == 1:
                    # emit the previous chunk's normalize only after this
                    # chunk's first tiles so its ACT ops (ln/exp) rank below
                    # the scores-exp in the scheduler's priority heap —
                    # keeps the PE fed at chunk boundaries
                    emit_normalize(p, pending[0], pending[1])
                    pending = None
            pending = (c, avs)
        emit_normalize(p, pending[0], pending[1])
        # exchange this head-pair as soon as it is complete; the first
        # AllToAll overlaps with the second pair's attention compute
        nc.gpsimd.collective_compute(
            "AllToAll",
            mybir.AluOpType.bypass,
            replica_groups=[list(range(NCORES))],
            ins=[a2a_in[p][:].opt()],
            outs=[a2a_out[p][:].opt()],
        )

    # ---------------- Phase D: out projection ----------------
    # Split by head-pair parity: the pair-0 (even c-chunk) half of the
    # accumulation runs as soon as A2A#0 lands — i.e. under the exposed
    # A2A#1 window — into SBUF partials; the pair-1 half accumulates after
    # A2A#1 and the sum is written out.
    aoT = {}
    for par in range(2):
        for cb in range(par, 8, 2):  # c-chunk cb = 2*(group) + pair
            t = persist.tile([128, QCH], bf16, name=f"aoT{cb}_{it}",
                             tag=f"aoT{cb}")
            src = a2a_out[par][:][bass.ds(blk_sv + (cb // 2), 1), :, :]
            nc.gpsimd.dma_start(
                out=t[:],
                in_=src.rearrange("b p n -> p b n").opt(keep_dims={0}))
            aoT[cb] = t
        if par == 0:
            o_part = []
            for t4 in range(4):
                op_t = osb.tile([128, DIM_K], f32, tag="osb",
                                name=f"opart{t4}_{it}")
                o_part.append(op_t)
                for oc in range(2):
                    ps = ps_big.tile([128, QCH], f32, tag="big",
                                     name=f"ops0_{t4}_{oc}_{it}")
                    for k2, cb in enumerate(range(0, 8, 2)):
                        nc.tensor.matmul(
                            ps[:],
                            lhsT=aoT[cb][:, 128 * t4:128 * (t4 + 1)],
                            rhs=wo_sb[:, cb, QCH * oc:QCH * (oc + 1)],
                            start=(k2 == 0), stop=(k2 == 3),
                        )
                    nc.vector.tensor_copy(
                        op_t[:, QCH * oc:QCH * (oc + 1)], ps[:])
        else:
            for t4 in range(4):
                for oc in range(2):
                    ps = ps_big.tile([128, QCH], f32, tag="big",
                                     name=f"ops1_{t4}_{oc}_{it}")
                    for k2, cb in enumerate(range(1, 8, 2)):
                        nc.tensor.matmul(
                            ps[:],
                            lhsT=aoT[cb][:, 128 * t4:128 * (t4 + 1)],
                            rhs=wo_sb[:, cb, QCH * oc:QCH * (oc + 1)],
                            start=(k2 == 0), stop=(k2 == 3),
                        )
                    nc.vector.tensor_add(
                        o_part[t4][:, QCH * oc:QCH * (oc + 1)],
                        o_part[t4][:, QCH * oc:QCH * (oc + 1)],
                        ps[:])
                    nc.sync.dma_start(
                        out=out[128 * t4:128 * (t4 + 1),
                                QCH * oc:QCH * (oc + 1)],
                        in_=o_part[t4][:, QCH * oc:QCH * (oc + 1)])


def _build(dup=1):
    import concourse.tile as tile
    from concourse import bacc, mybir
    import concourse.bacc as bacc_mod
    from concourse.hw_specs import get_activation_tables as _orig_tables

    # This kernel only uses Exp and Ln, and both live in the
    # natural_log_exp_and_others table at full resolution. Hide them from
    # every other table (dict order/positions preserved) so the table-load
    # pass assigns one table for the whole kernel instead of thrashing
    # Exp<->Ln tables (~1.3us per reload, twice per q-chunk) on ACT.
    _EXP = mybir.ActivationFunctionType.Exp
    _LN = mybir.ActivationFunctionType.Ln

    def _patched_tables(arch):
        t = {k: set(v) for k, v in _orig_tables(arch).items()}
        for name, funcs in t.items():
            if name != "natural_log_exp_and_others":
                funcs.discard(_EXP)
                funcs.discard(_LN)
        return t

    bacc_mod.get_activation_tables = _patched_tables

    f32 = mybir.dt.float32
    f32r = mybir.dt.float32r
    bf16 = mybir.dt.bfloat16

    nc = bacc.Bacc("TRN2", target_bir_lowering=False, debug=False,
                   num_devices=NCORES)

    x_in = nc.dram_tensor("x", [D, S], bf16, kind="ExternalInput")  # x^T
    wq_in = nc.dram_tensor("wq", [128, NDC, C], bf16, kind="ExternalInput")
    wk_in = nc.dram_tensor("wk", [128, NDC, C], bf16, kind="ExternalInput")
    wv_in = nc.dram_tensor("wv", [128, NDC, C], bf16, kind="ExternalInput")
    wo_in = nc.dram_tensor("wo", [128, NDC, DIM_K], bf16, kind="ExternalInput")
    tri_in = nc.dram_tensor("trimask", [128, 128], bf16, kind="ExternalInput")
    info_in = nc.dram_tensor("coreinfo", [1, 2], mybir.dt.uint32,
                             kind="ExternalInput")
    out = nc.dram_tensor("out", [QCH, DIM_K], f32, kind="ExternalOutput")
    ins = (x_in, wq_in, wk_in, wv_in, wo_in, tri_in, info_in, out)

    with tile.TileContext(nc) as tc:
        with (
            tc.tile_pool(name="persist", bufs=1) as persist,
            tc.tile_pool(name="exps", bufs=6) as exps,
            tc.tile_pool(name="aop", bufs=4) as aop,
            tc.tile_pool(name="recips", bufs=2) as recips,
            tc.tile_pool(name="osb", bufs=4) as osb,
            tc.tile_pool(name="ps_big", bufs=2, space="PSUM") as ps_big,
            tc.tile_pool(name="ps_av", bufs=4, space="PSUM") as ps_av,
            tc.tile_pool(name="dram", bufs=1, space="DRAM") as dram,
        ):
            pools = (persist, exps, aop, recips, osb, ps_big, ps_av, dram)
            for it in range(dup):
                _emit_body(nc, tc, pools, ins, it)

    nc.compile()
    return nc


def _get_nc(dup=1):
    key = f"nc{dup}"
    if key not in _cache:
        _cache[key] = _build(dup)
    return _cache[key]


def _shuf(w):
    # [D_in, D_out] -> [128, D_in//128, D_out] partition-major
    return np.ascontiguousarray(
        w.reshape(NDC, 128, w.shape[1]).transpose(1, 0, 2))


def _make_in_maps(x, Wq, Wk, Wv, Wo):
    bf = ml_dtypes.bfloat16
    x_bf = np.asarray(x, np.float32).astype(bf)       # [B, S, D]
    xt_bf = [np.ascontiguousarray(x_bf[b].T) for b in range(B)]
    wq_bf = np.asarray(Wq, np.float32).astype(bf)
    wk_bf = np.asarray(Wk, np.float32).astype(bf)
    wv_bf = np.asarray(Wv, np.float32).astype(bf)
    wo_sh = _shuf(np.asarray(Wo, np.float32).astype(bf))
    tri = np.triu(np.ones((128, 128), np.float32)).astype(bf)

    in_maps = []
    for c in range(NCORES):
        b, g = divmod(c, HC)
        cols = slice(C * g, C * (g + 1))
        info = np.array([[HC * b, QCH * g]], dtype=np.uint32)
        in_maps.append({
            "x": xt_bf[b],
            "wq": _shuf(wq_bf[:, cols]),
            "wk": _shuf(wk_bf[:, cols]),
            "wv": _shuf(wv_bf[:, cols]),
            "wo": wo_sh,
            "trimask": tri,
            "coreinfo": info,
        })
    return in_maps


def kernel(x, Wq, Wk, Wv, Wo, _dup=1, _trace=False, _trace_kwargs=None):
    from concourse.bass_utils import run_bass_kernel_spmd

    in_maps = _make_in_maps(x, Wq, Wk, Wv, Wo)
    nc = _get_nc(_dup)
    res = run_bass_kernel_spmd(
        nc, in_maps, list(range(NCORES)),
        trace=_trace, **(_trace_kwargs or {}))
    _cache["last_result"] = res

    outp = np.empty((B, S, DIM_K), np.float32)
    for c in range(NCORES):
        b, g = divmod(c, HC)
        outp[b, QCH * g:QCH * (g + 1), :] = res.results[c]["out"]
    return outp


# revision 15
# speedup vs baseline: 1.3158x; 1.3158x over previous
"""Multi-head causal attention on 8 TRN2 NeuronCores.

Sharding: (batch, head-group) across 8 cores — core c handles batch c//4 and
heads [4*(c%4), 4*(c%4)+4). After attention, an 8-rank AllToAll exchanges
per-head attention outputs so core c computes the final output projection for
rows [512*(c%4), 512*(c%4)+512) of batch c//4. Host-side unshard is a pure
concatenation.

All matmuls run in bf16 (fp32 PSUM accumulation). Softmax is computed without
max-subtraction (scores*scale are O(1) for these inputs); the denominator is
obtained by augmenting V with a ones column. The division uses the fast
approximate DVE reciprocal plus a GpSimd partition-broadcast and one vector
multiply — cheap enough to stay off the PE critical path.
"""
import numpy as np
import ml_dtypes

B, S, D, H = 2, 2048, 1024, 16
DH = D // H          # 64
DIM_K = 1024
NCORES = 8
HC = 4               # heads per core
C = HC * DH          # 256 dh-columns per core
NQC = 4              # q-chunks of 512
QCH = 512
NKT = 16             # k-tiles of 128
NDC = 8              # d-chunks of 128
SCALE = float(DIM_K) ** -0.5  # 1/32

_cache = {}


def _emit_body(nc, tc, pools, ins, it):
    """Emit one full kernel body (iteration `it` for duplication timing)."""
    import concourse.bass as bass
    from concourse import mybir

    f32 = mybir.dt.float32
    f32r = mybir.dt.float32r
    bf16 = mybir.dt.bfloat16
    EXP = mybir.ActivationFunctionType.Exp
    LN = mybir.ActivationFunctionType.Ln

    persist, exps, aop, recips, osb, ps_big, ps_av, dram = pools
    x_in, wq_in, wk_in, wv_in, wo_in, tri_in, info_in, out = ins

    # ---------------- Phase A: loads ----------------
    # Host passes partition-major layouts, so every load is one contiguous
    # DMA. Small QKV weights first (they gate the first matmuls); x comes in
    # per-512-column chunks so chunk-0 projections can start early; Wo last.
    wq_sb = persist.tile([128, NDC, C], bf16, name=f"wq_sb_{it}", tag="wq_sb")
    wk_sb = persist.tile([128, NDC, C], bf16, name=f"wk_sb_{it}", tag="wk_sb")
    wv_sb = persist.tile([128, NDC, C], bf16, name=f"wv_sb_{it}", tag="wv_sb")
    wo_sb = persist.tile([128, NDC, DIM_K], bf16, name=f"wo_sb_{it}", tag="wo_sb")
    nc.sync.dma_start(out=wq_sb[:], in_=wq_in.ap())
    nc.sync.dma_start(out=wk_sb[:], in_=wk_in.ap())
    nc.sync.dma_start(out=wv_sb[:], in_=wv_in.ap())

    xT = [persist.tile([128, S], bf16, name=f"xT{j}_{it}", tag=f"xT{j}")
          for j in range(NDC)]
    for j in range(NDC):
        nc.sync.dma_start(out=xT[j][:, 0:QCH],
                          in_=x_in[128 * j:128 * (j + 1), 0:QCH])

    tri = persist.tile([128, 128], bf16, name=f"tri_{it}", tag="tri")
    nc.sync.dma_start(out=tri[:], in_=tri_in.ap())

    for qc in range(1, NQC):
        for j in range(NDC):
            nc.sync.dma_start(
                out=xT[j][:, QCH * qc:QCH * (qc + 1)],
                in_=x_in[128 * j:128 * (j + 1), QCH * qc:QCH * (qc + 1)])

    nc.sync.dma_start(out=wo_sb[:], in_=wo_in.ap())

    # ---------------- Phase B: QKV projections ----------------
    # Q^T / K^T in pair tiles: [128, S], heads (2p, 2p+1) at partitions
    # [0,64) / [64,128).
    qt, kt = [None, None], [None, None]

    def emit_qtkt(p):
        qtp = persist.tile([128, S], bf16, name=f"qt{p}_{it}", tag=f"qt{p}")
        ktp = persist.tile([128, S], bf16, name=f"kt{p}_{it}", tag=f"kt{p}")
        qt[p] = qtp
        kt[p] = ktp
        for w_sb, dst in ((wq_sb, qtp), (wk_sb, ktp)):
            for qc in range(NQC):
                ps = ps_big.tile([128, QCH], f32, tag="big",
                                 name=f"qkps{p}_{qc}_{w_sb.name[:2]}_{it}")
                for j in range(NDC):
                    nc.tensor.matmul(
                        ps[:],
                        lhsT=w_sb[:, j, 128 * p:128 * (p + 1)],
                        rhs=xT[j][:, QCH * qc:QCH * (qc + 1)],
                        start=(j == 0), stop=(j == NDC - 1),
                    )
                nc.vector.tensor_copy(dst[:, QCH * qc:QCH * (qc + 1)], ps[:])

    emit_qtkt(0)

    # V padded to 128 columns: [ones | 63 zeros | 64 data] per head. The
    # ones column at index 0 puts the softmax denominator in PSUM row 0
    # (partition_broadcast can only read a partition-0 source) and the
    # data rows at partitions 64..127 (engine APs need aligned bases).
    # 128-wide bf16 weights also enable FWL fast weight loads on the PE.
    vp = []
    for i in range(NKT):
        t = persist.tile([128, HC, 2 * DH], bf16, name=f"vp{i}_{it}",
                         tag=f"vp{i}")
        nc.vector.memset(t[:, :, 0:DH], 0.0)
        nc.vector.memset(t[:, :, 0:1], 1.0)
        ps = ps_big.tile([128, C], f32, tag="big", name=f"vps{i}_{it}")
        for j in range(NDC):
            nc.tensor.matmul(
                ps[:],
                lhsT=xT[j][:, 128 * i:128 * (i + 1)],
                rhs=wv_sb[:, j, :],
                start=(j == 0), stop=(j == NDC - 1),
            )
        nc.vector.tensor_copy(
            t[:, :, DH:2 * DH], ps[:].rearrange("p (h d) -> p h d", h=HC))
        vp.append(t)

    # pair-1 projections emitted here so the scheduler can fill PE gaps
    # during pair-0's (ACT-bound) attention with these matmuls
    emit_qtkt(1)

    # ---------------- Phase C: attention ----------------
    # Per head-pair AllToAll buffers: block j carries my pair-p rows for
    # rank j's s-block. I fill only blocks [4b, 4b+4) (my batch's ranks);
    # 4b comes from coreinfo at runtime.
    blk = nc.gpsimd.alloc_register(f"blk_{it}")
    nc.gpsimd.reg_load(blk, info_in[0:1, 0:1])
    blk_sv = nc.gpsimd.snap(blk, donate=True, min_val=0, max_val=NCORES - HC)

    a2a_in = [dram.tile([NCORES, 128, QCH], bf16, name=f"a2a_in{p}_{it}",
                        tag=f"a2a_in{p}") for p in range(2)]
    a2a_out = [dram.tile([NCORES, 128, QCH], bf16, name=f"a2a_out{p}_{it}",
                         tag=f"a2a_out{p}") for p in range(2)]
    def emit_normalize(p, c, avs):
        for h2 in range(2):
            # 1/denom = exp(-ln(denom)) on the ACT LUT engine; the
            # broadcast across partitions runs on GpSimd. The PE and
            # DVE stay out of the softmax-normalize chain entirely.
            lnd = recips.tile([1, QCH], f32, tag="lnd",
                              name=f"lnd{p}_{c}_{h2}_{it}")
            nc.scalar.activation(out=lnd[:], in_=avs[h2][0:1, :],
                                 func=LN)
            rc = recips.tile([1, QCH], f32, tag="rc",
                             name=f"rc{p}_{c}_{h2}_{it}")
            nc.scalar.activation(out=rc[:], in_=lnd[:], func=EXP,
                                 scale=-1.0)
            bc_sb = recips.tile([128, QCH], f32, tag="bcsb",
                                name=f"bcsb{p}_{c}_{h2}_{it}")
            nc.gpsimd.partition_broadcast(bc_sb[:], rc[0:1, :])
            ao = aop.tile([128, QCH], bf16, tag="ao",
                          name=f"ao{p}_{c}_{h2}_{it}")
            nc.vector.tensor_mul(ao[DH:2 * DH, :],
                                 avs[h2][DH:2 * DH, :],
                                 bc_sb[DH:2 * DH, :])
            # static writes to both batches' candidate blocks (c, c+4);
            # the wrong-batch block is ignored by its receiver
            for bb in range(2):
                nc.sync.dma_start(
                    out=a2a_in[p][HC * bb + c, DH * h2:DH * (h2 + 1), :],
                    in_=ao[DH:2 * DH, :])

    for p in range(2):
        for c in range(NQC):
            avs = [ps_av.tile([128, QCH], f32, tag="av",
                              name=f"av{p}_{c}_{i2}_{it}")
                   for i2 in range(2)]
            njt = 4 * c + 4
            for j in range(njt):
                off = max(0, 128 * j - QCH * c)
                sc = ps_big.tile([128, 2 * QCH], f32, tag="big",
                                 name=f"sc{p}_{c}_{j}_{it}")
                sc3 = sc[:].rearrange("p (h n) -> p h n", h=2)
                ex = exps.tile([128, 2, QCH], bf16, tag="ex",
                               name=f"ex{p}_{c}_{j}_{it}")
                for h2 in range(2):
                    nc.tensor.matmul(
                        sc3[:, h2, off:QCH],
                        lhsT=kt[p][64 * h2:64 * (h2 + 1), 128 * j:128 * (j + 1)],
                        rhs=qt[p][64 * h2:64 * (h2 + 1),
                                  QCH * c + off:QCH * (c + 1)],
                        start=True, stop=True,
                    )
                nc.scalar.activation(
                    out=ex[:, :, off:QCH], in_=sc3[:, :, off:QCH],
                    func=EXP, scale=SCALE)
                if j // 4 == c:
                    # diagonal tile: zero the strictly-lower triangle
                    nc.vector.tensor_mul(
                        ex[:, :, off:off + 128],
                        ex[:, :, off:off + 128],
                        tri[:].unsqueeze(1).to_broadcast([128, 2, 128]),
                    )
                for h2 in range(2):
                    nc.tensor.matmul(
                        avs[h2][:, off:QCH],
                        lhsT=vp[j][:, 2 * p + h2, :],
                        rhs=ex[:, h2, off:QCH],
                        start=(j == 0), stop=(j == njt - 1),
                    )
            emit_normalize(p, c, avs)
        # exchange this head-pair as soon as it is complete; the first
        # AllToAll overlaps with the second pair's attention compute
        nc.gpsimd.collective_compute(
            "AllToAll",
            mybir.AluOpType.bypass,
            replica_groups=[list(range(NCORES))],
            ins=[a2a_in[p][:].opt()],
            outs=[a2a_out[p][:].opt()],
        )

    # ---------------- Phase D: out projection ----------------
    # Split by head-pair parity: the pair-0 (even c-chunk) half of the
    # accumulation runs as soon as A2A#0 lands — i.e. under the exposed
    # A2A#1 window — into SBUF partials; the pair-1 half accumulates after
    # A2A#1 and the sum is written out.
    aoT = {}
    for par in range(2):
        for cb in range(par, 8, 2):  # c-chunk cb = 2*(group) + pair
            t = persist.tile([128, QCH], bf16, name=f"aoT{cb}_{it}",
                             tag=f"aoT{cb}")
            src = a2a_out[par][:][bass.ds(blk_sv + (cb // 2), 1), :, :]
            nc.gpsimd.dma_start(
                out=t[:],
                in_=src.rearrange("b p n -> p b n").opt(keep_dims={0}))
            aoT[cb] = t
        if par == 0:
            o_part = []
            for t4 in range(4):
                op_t = osb.tile([128, DIM_K], f32, tag="osb",
                                name=f"opart{t4}_{it}")
                o_part.append(op_t)
                for oc in range(2):
                    ps = ps_big.tile([128, QCH], f32, tag="big",
                                     name=f"ops0_{t4}_{oc}_{it}")
                    for k2, cb in enumerate(range(0, 8, 2)):
                        nc.tensor.matmul(
                            ps[:],
                            lhsT=aoT[cb][:, 128 * t4:128 * (t4 + 1)],
                            rhs=wo_sb[:, cb, QCH * oc:QCH * (oc + 1)],
                            start=(k2 == 0), stop=(k2 == 3),
                        )
                    nc.vector.tensor_copy(
                        op_t[:, QCH * oc:QCH * (oc + 1)], ps[:])
        else:
            for t4 in range(4):
                for oc in range(2):
                    ps = ps_big.tile([128, QCH], f32, tag="big",
                                     name=f"ops1_{t4}_{oc}_{it}")
                    for k2, cb in enumerate(range(1, 8, 2)):
                        nc.tensor.matmul(
                            ps[:],
                            lhsT=aoT[cb][:, 128 * t4:128 * (t4 + 1)],
                            rhs=wo_sb[:, cb, QCH * oc:QCH * (oc + 1)],
                            start=(k2 == 0), stop=(k2 == 3),
                        )
                    nc.vector.tensor_add(
                        o_part[t4][:, QCH * oc:QCH * (oc + 1)],
                        o_part[t4][:, QCH * oc:QCH * (oc + 1)],
                        ps[:])
                    nc.sync.dma_start(
                        out=out[128 * t4:128 * (t4 + 1),
                                QCH * oc:QCH * (oc + 1)],
                        in_=o_part[t4][:, QCH * oc:QCH * (oc + 1)])


def _build(dup=1):
    import concourse.tile as tile
    from concourse import bacc, mybir
    import concourse.bacc as bacc_mod
    from concourse.hw_specs import get_activation_tables as _orig_tables

    # This kernel only uses Exp and Ln, and both live in the
    # natural_log_exp_and_others table at full resolution. Hide them from
    # every other table (dict order/positions preserved) so the table-load
    # pass assigns one table for the whole kernel instead of thrashing
    # Exp<->Ln tables (~1.3us per reload, twice per q-chunk) on ACT.
    _EXP = mybir.ActivationFunctionType.Exp
    _LN = mybir.ActivationFunctionType.Ln

    def _patched_tables(arch):
        t = {k: set(v) for k, v in _orig_tables(arch).items()}
        for name, funcs in t.items():
            if name != "natural_log_exp_and_others":
                funcs.discard(_EXP)
                funcs.discard(_LN)
        return t

    bacc_mod.get_activation_tables = _patched_tables

    f32 = mybir.dt.float32
    f32r = mybir.dt.float32r
    bf16 = mybir.dt.bfloat16

    nc = bacc.Bacc("TRN2", target_bir_lowering=False, debug=False,
                   num_devices=NCORES)

    x_in = nc.dram_tensor("x", [D, S], bf16, kind="ExternalInput")  # x^T
    wq_in = nc.dram_tensor("wq", [128, NDC, C], bf16, kind="ExternalInput")
    wk_in = nc.dram_tensor("wk", [128, NDC, C], bf16, kind="ExternalInput")
    wv_in = nc.dram_tensor("wv", [128, NDC, C], bf16, kind="ExternalInput")
    wo_in = nc.dram_tensor("wo", [128, NDC, DIM_K], bf16, kind="ExternalInput")
    tri_in = nc.dram_tensor("trimask", [128, 128], bf16, kind="ExternalInput")
    info_in = nc.dram_tensor("coreinfo", [1, 2], mybir.dt.uint32,
                             kind="ExternalInput")
    out = nc.dram_tensor("out", [QCH, DIM_K], f32, kind="ExternalOutput")
    ins = (x_in, wq_in, wk_in, wv_in, wo_in, tri_in, info_in, out)

    with tile.TileContext(nc) as tc:
        with (
            tc.tile_pool(name="persist", bufs=1) as persist,
            tc.tile_pool(name="exps", bufs=6) as exps,
            tc.tile_pool(name="aop", bufs=4) as aop,
            tc.tile_pool(name="recips", bufs=2) as recips,
            tc.tile_pool(name="osb", bufs=4) as osb,
            tc.tile_pool(name="ps_big", bufs=2, space="PSUM") as ps_big,
            tc.tile_pool(name="ps_av", bufs=4, space="PSUM") as ps_av,
            tc.tile_pool(name="dram", bufs=1, space="DRAM") as dram,
        ):
            pools = (persist, exps, aop, recips, osb, ps_big, ps_av, dram)
            for it in range(dup):
                _emit_body(nc, tc, pools, ins, it)

    nc.compile()
    return nc


def _get_nc(dup=1):
    key = f"nc{dup}"
    if key not in _cache:
        _cache[key] = _build(dup)
    return _cache[key]


def _shuf(w):
    # [D_in, D_out] -> [128, D_in//128, D_out] partition-major
    return np.ascontiguousarray(
        w.reshape(NDC, 128, w.shape[1]).transpose(1, 0, 2))


def _make_in_maps(x, Wq, Wk, Wv, Wo):
    bf = ml_dtypes.bfloat16
    x_bf = np.asarray(x, np.float32).astype(bf)       # [B, S, D]
    xt_bf = [np.ascontiguousarray(x_bf[b].T) for b in range(B)]
    wq_bf = np.asarray(Wq, np.float32).astype(bf)
    wk_bf = np.asarray(Wk, np.float32).astype(bf)
    wv_bf = np.asarray(Wv, np.float32).astype(bf)
    wo_sh = _shuf(np.asarray(Wo, np.float32).astype(bf))
    tri = np.triu(np.ones((128, 128), np.float32)).astype(bf)

    in_maps = []
    for c in range(NCORES):
        b, g = divmod(c, HC)
        cols = slice(C * g, C * (g + 1))
        info = np.array([[HC * b, QCH * g]], dtype=np.uint32)
        in_maps.append({
            "x": xt_bf[b],
            "wq": _shuf(wq_bf[:, cols]),
            "wk": _shuf(wk_bf[:, cols]),
            "wv": _shuf(wv_bf[:, cols]),
            "wo": wo_sh,
            "trimask": tri,
            "coreinfo": info,
        })
    return in_maps


def kernel(x, Wq, Wk, Wv, Wo, _dup=1, _trace=False, _trace_kwargs=None):
    from concourse.bass_utils import run_bass_kernel_spmd

    in_maps = _make_in_maps(x, Wq, Wk, Wv, Wo)
    nc = _get_nc(_dup)
    res = run_bass_kernel_spmd(
        nc, in_maps, list(range(NCORES)),
        trace=_trace, **(_trace_kwargs or {}))
    _cache["last_result"] = res

    outp = np.empty((B, S, DIM_K), np.float32)
    for c in range(NCORES):
        b, g = divmod(c, HC)
        outp[b, QCH * g:QCH * (g + 1), :] = res.results[c]["out"]
    return outp
